# revision 17
# baseline (speedup 1.0000x reference)
"""Trainium2 Bass kernel for nn_DecoderHead (B=2, T=2048, D=1024, H=16, DH=64).

y = x + softmax_causal((x @ Wq.T) split to heads @ k^T / sqrt(D)) @ v

Sharding: 8 cores = 2 (batch) x 4 (head groups of 4 heads). Each core computes
its batch's q-projection for its 256 output features (Wq column-sharded by
head), causal attention for its 4 heads, adds the residual slice, and writes a
[T, 256] slice; the host concatenates slices (the all-gather over the
head-split d dim is a free host-side assembly).

Per-core dataflow (all matmul contractions on the PE partition axis):
  qT[e, t]   = sum_d WqT[d, e] * xT[d, t]         (q projection, transposed)
  sT[tk, tq] = sum_dh kT_h[dh, tk] * qT_h[dh, tq] (scores, transposed; the two
               heads of a pair are emitted INTERLEAVED so the PE runs them
               concurrently in distinct 64-row groups -> 2x score rate)
  eT         = exp(sT / 32), causal-masked        (ACT exp on the visible
               column range only; DVE mask-mul only on the two mixed
               128-col diagonal blocks of each diagonal pair)
  oT[dh', tq]= sum_tk vO[tk, dh'] * eT[tk, tq]    (vO = [v | ones]; row 64
               accumulates the denominator; streams only visible columns)
  y[tq, dh]  = transpose(oT) / denom + x_res      (PE transpose into one PSUM
                                                   bank, fused DVE epilogue)

All matmul operands are bf16 (fp32 PSUM accumulate): standalone LDWEIGHTS +
FWL make the weight loads hide behind the streams, and row-group pairing
works (fp32r self-loading matmuls serialize the array).
"""

import os
from collections import deque

import numpy as np

import concourse.bass as bass
import concourse.mybir as mybir
import concourse.tile as tile
from concourse import bacc
from concourse.alu_op_type import AluOpType
from concourse.bass_utils import run_bass_kernel_spmd

# Problem shape (hardcoded per the harness contract).
B, T, D, H = 2, 2048, 1024, 16
DH = D // H          # 64
N_CORES = 8
HPC = H // (N_CORES // B)   # heads per core = 4
EPC = HPC * DH       # output features per core = 256
P = 128              # SBUF partitions
TQ = 512             # query-tile width (matmul moving-dim)
NTQ = T // TQ        # 4
NTKB = T // P        # 16 key blocks of 128
DT = D // P          # 8 contraction tiles for the q projection
EG = EPC // P        # 2 head-pair groups of 128 e-rows
SCALE = 1.0 / np.sqrt(np.float32(D))   # 1/32 (reference scales by sqrt(d))

F32 = mybir.dt.float32
FP8 = mybir.dt.float8e4
U8 = mybir.dt.uint8
I8 = mybir.dt.int8
VPAD = 80            # vO innermost pad: DoubleRow LDW needs Ko step % 16 == 0
LOG2E = 1.4426950408889634
# Schraudolph exp for fp8e4m3 out: bits = round(8*(x*SCALE*log2e - c) + 56)
SCH_A = float(1.0 / np.sqrt(np.float32(D))) * LOG2E * 8.0
SCH_B = 56.0 - 8.0 * 0.0430

# Matmul operand dtype: bf16 (default; full PE rate + hideable weight loads),
# fp32r (fp32 w/ 11-bit mantissa), fp32 (exact, 1/4 rate).
VARIANT = os.environ.get("DH_VARIANT", "bf16")
# Route head-0 non-diagonal exps to the DVE (Schraudolph bit-trick) to
# offload the saturated ACT engine.
DVE_EXP = os.environ.get("DH_DVE_EXP", "1") == "1"


def _mm_dt(variant):
    return {
        "fp32": mybir.dt.float32,
        "fp32r": mybir.dt.float32r,
        "bf16": mybir.dt.bfloat16,
    }[variant]


def _np_round_fp32r(a: np.ndarray) -> np.ndarray:
    """Round fp32 to the fp32r value set: 11-bit mantissa, RNE, low 12 bits 0."""
    u = a.astype(np.float32).view(np.uint32)
    lsb = (u >> np.uint32(12)) & np.uint32(1)
    r = (u + np.uint32(0x7FF) + lsb) & np.uint32(0xFFFFF000)
    return r.view(np.float32)


def _host_cast(a: np.ndarray, variant: str) -> np.ndarray:
    a = np.ascontiguousarray(a, dtype=np.float32)
    if variant == "fp32r":
        return _np_round_fp32r(a)
    if variant == "bf16":
        import ml_dtypes
        return a.astype(ml_dtypes.bfloat16)
    return a


def build_nc(variant: str = VARIANT, repeat: int = 1):
    """Build the per-core SPMD Bass program. `repeat` wraps the body in a
    hardware loop (timing only)."""
    mdt = _mm_dt(variant)
    # oT / identity dtype: the PE transpose requires out/lhsT dtype match and
    # pst is fp32 PSUM, so these stay fp32 in every variant.
    odt = F32
    nc = bacc.Bacc(
        "TRN2", target_bir_lowering=False, debug=False, num_devices=N_CORES
    )

    xT = nc.dram_tensor("xT", [D, T], mdt, kind="ExternalInput").ap()
    wqT = nc.dram_tensor("wqT", [D, EPC], mdt, kind="ExternalInput").ap()
    kT = nc.dram_tensor("kT", [P, EG, T], mdt, kind="ExternalInput").ap()
    vO = nc.dram_tensor("vO", [P, NTKB, HPC, VPAD], FP8, kind="ExternalInput").ap()
    xres = nc.dram_tensor("xres", [P, T // P, EPC], F32, kind="ExternalInput").ap()
    # mask2[k, u, j]: causal mask (0x00/0xFF bytes, ANDed onto fp8 et) for the
    # 256-wide mixed window of a diagonal pair: u=0 block -> (k <= j) for
    # j<128 else 1; u=1 block -> 0 for j<128 else (k <= j-128).
    mask2 = nc.dram_tensor("mask2", [P, 2, 2 * P], U8, kind="ExternalInput").ap()
    ident = nc.dram_tensor("ident", [P, P], F32, kind="ExternalInput").ap()
    y = nc.dram_tensor("y", [T, EPC], F32, kind="ExternalOutput").ap()

    with tile.TileContext(nc) as tc:
        with (
            tc.tile_pool(name="const", bufs=1) as cpool,
            tc.tile_pool(name="xq", bufs=1) as xqpool,
            tc.tile_pool(name="work", bufs=6) as wpool,
            tc.tile_pool(name="epi", bufs=2) as epool,
            tc.tile_pool(name="ps_s", bufs=3, space="PSUM") as ps_s,
            tc.tile_pool(name="ps_o", bufs=2, space="PSUM") as ps_o,
        ):
            def body(_iv=None):
                # ---- tiles -------------------------------------------------
                id_sb = cpool.tile([P, P], F32, name="id_sb", tag="id_sb")
                mk_sb = cpool.tile([P, 2, 2 * P], U8, name="mk_sb", tag="mk_sb")
                wq_sb = xqpool.tile([P, DT, EPC], mdt, name="wq_sb", tag="wq_sb")
                xT_sb = xqpool.tile([P, DT, T], mdt, name="xT_sb", tag="xT_sb")
                kT_sb = cpool.tile([P, EG, T], mdt, name="kT_sb", tag="kT_sb")
                vO_sb = cpool.tile([P, NTKB, HPC, VPAD], FP8, name="vO_sb",
                                   tag="vO_sb")
                xr_sb = cpool.tile([P, T // P, EPC], F32, name="xr_sb",
                                   tag="xr_sb")
                qT_sb = xqpool.tile([P, EG, T], mdt, name="qT_sb", tag="qT_sb")

                # ---- stage-0 loads ----------------------------------------
                nc.sync.dma_start(id_sb[:], ident[:])
                for dt_i in range(DT):
                    nc.sync.dma_start(
                        wq_sb[:, dt_i, :], wqT[dt_i * P:(dt_i + 1) * P, :]
                    )

                def load_stage(c):
                    """Inputs first needed by tq-tile c."""
                    sl = bass.ts(c, TQ)
                    for dt_i in range(DT):
                        nc.sync.dma_start(
                            xT_sb[:, dt_i, sl], xT[dt_i * P:(dt_i + 1) * P, sl]
                        )
                    for g in range(EG):
                        nc.sync.dma_start(kT_sb[:, g, sl], kT[:, g, sl])
                    nc.sync.dma_start(
                        vO_sb[:, 4 * c:4 * (c + 1)], vO[:, 4 * c:4 * (c + 1)]
                    )
                    nc.sync.dma_start(
                        xr_sb[:, 4 * c:4 * (c + 1)], xres[:, 4 * c:4 * (c + 1)]
                    )

                load_stage(0)
                nc.sync.dma_start(mk_sb[:], mask2[:])

                # Warm-up while stage-0 DMA streams: prime the ACT exp table
                # and keep PE busy so the HAM clock-gate opens (dummy work on
                # the identity tile; results unused).
                warm_et = wpool.tile([P, P], F32, name="warm_et", tag="warm")
                psw = ps_o.tile([P, P], F32, name="psw", tag="o")
                for w in range(12):
                    nc.tensor.matmul(
                        psw[:], id_sb[:], id_sb[:], start=True, stop=True,
                    )
                nc.scalar.activation(
                    warm_et[:], psw[:],
                    mybir.ActivationFunctionType.Exp, scale=0.01,
                )

                pending = deque()

                def epilogue_start(h, tqt, pso_t):
                    oT = epool.tile([DH + 1, TQ], F32, name="oT", tag="oT",
                                    bufs=4)
                    nc.vector.tensor_copy(oT[:], pso_t[:])
                    return (h, tqt, oT)

                def epilogue(state):
                    h, tqt, oT = state
                    ysb = epool.tile([P, 4, DH], F32, name="ysb", tag="ysb")
                    pst = ps_o.tile([P, 4, DH + 1], F32, name="pst", tag="o")
                    for j in range(4):
                        nc.tensor.transpose(
                            pst[:, j, :],
                            oT[:, j * P:(j + 1) * P],
                            id_sb[0:DH + 1, 0:DH + 1],
                        )
                    rc = epool.tile([P, 4], F32, name="rc", tag="rc", bufs=4)
                    nc.vector.reciprocal(rc[:], pst[:, :, DH])
                    for j in range(4):
                        nc.vector.scalar_tensor_tensor(
                            ysb[:, j, :],
                            pst[:, j, 0:DH],
                            rc[:, j:j + 1],
                            xr_sb[:, 4 * tqt + j, h * DH:(h + 1) * DH],
                            AluOpType.mult,
                            AluOpType.add,
                        )
                    ydst = y[tqt * TQ:(tqt + 1) * TQ, h * DH:(h + 1) * DH]
                    nc.sync.dma_start(
                        ydst.rearrange("(j p) c -> p j c", p=P), ysb[:]
                    )

                def attention(hp, tqt):
                    g = hp
                    ntk = 4 * (tqt + 1)
                    npairs = ntk // 2
                    tq0 = tqt * TQ
                    pso2 = [
                        ps_o.tile([DH + 1, TQ], F32, name=f"pso{i}", tag="o")
                        for i in range(2)
                    ]

                    def vis_of(tkb):
                        # first visible query column (within the TQ tile) for
                        # key block tkb; columns below are fully masked.
                        return max(0, P * (tkb - 4 * tqt))

                    def emit_pv(p_et2, p_pair, last=False):
                        # One DoubleRow matmul per head covers both key blocks
                        # of the pair (Ko=2 contraction halves).
                        vis = vis_of(2 * p_pair)
                        for i in range(2):
                            nc.tensor.matmul(
                                pso2[i][:, vis:],
                                vO_sb[:, 2 * p_pair:2 * p_pair + 2,
                                      2 * hp + i, 0:DH + 1],
                                p_et2[i][:, :, vis:],
                                start=(p_pair == 0),
                                stop=last,
                                perf_mode=mybir.MatmulPerfMode.DoubleRow,
                                skip_group_check=True,
                            )

                    prev = None
                    for pair in range(npairs):
                        diag = 2 * pair >= 4 * tqt   # this pair straddles the
                        m0 = 2 * pair - 4 * tqt      # causal diagonal
                        vis0 = vis_of(2 * pair)
                        et2 = []
                        pssc2 = [
                            ps_s.tile([P, 2, TQ], F32, name=f"pssc{i}", tag="s")
                            for i in range(2)
                        ]
                        # scores: interleave the two heads (i) inside the key
                        # block loop (u) so adjacent matmuls target distinct
                        # 64-row PE groups and run concurrently.
                        for u in range(2):
                            tkb = 2 * pair + u
                            vis = vis_of(tkb)
                            for i in range(2):
                                bp = DH * i
                                nc.tensor.matmul(
                                    pssc2[i][:, u, vis:],
                                    kT_sb[bp:bp + DH, g,
                                          tkb * P:(tkb + 1) * P],
                                    qT_sb[bp:bp + DH, g, tq0 + vis:tq0 + TQ],
                                    start=True,
                                    stop=True,
                                    skip_group_check=True,
                                )
                        for i in range(2):
                            et = wpool.tile([P, 2, TQ], FP8,
                                            name=f"et{i}", tag="et")
                            # NOTE: exp covers [vis0:] for BOTH u (the u=1
                            # block's extra 128 cols are zeroed by the mask)
                            # so the DoubleRow stream never reads garbage.
                            if DVE_EXP and not diag and i == 0:
                                nc.vector.tensor_scalar(
                                    et[:].bitcast(I8), pssc2[i][:],
                                    SCH_A, SCH_B,
                                    AluOpType.mult, AluOpType.add,
                                )
                            else:
                                nc.scalar.activation(
                                    et[:, :, vis0:], pssc2[i][:, :, vis0:],
                                    mybir.ActivationFunctionType.Exp,
                                    scale=float(SCALE),
                                )
                            if diag:
                                # AND-mask the 256-wide mixed window (covers
                                # the two mixed 128-col diagonal blocks and
                                # zeroes the u=1 head-start strip). Runs on
                                # the otherwise-idle GpSimd engine.
                                w0 = P * m0
                                nc.vector.tensor_tensor(
                                    et[:, :, w0:w0 + 2 * P].bitcast(U8),
                                    et[:, :, w0:w0 + 2 * P].bitcast(U8),
                                    mk_sb[:],
                                    AluOpType.bitwise_and,
                                )
                            et2.append(et)
                        if prev is not None:
                            emit_pv(*prev)
                        prev = (et2, pair)
                        if pending and pair < 2:
                            epilogue(pending.popleft())
                    emit_pv(*prev, last=True)
                    for i in range(2):
                        pending.append(epilogue_start(2 * hp + i, tqt, pso2[i]))

                # ---- main schedule: staged loads; qproj for tile c+1 is
                # emitted between the two head-pairs of attention tile c so
                # its matmuls fill PE stall slots while ACT/DVE chew on exp.
                def qproj(tqc):
                    sl = bass.ts(tqc, TQ)
                    for g in range(EG):
                        psq = ps_s.tile([P, TQ], F32, name="psq", tag="s")
                        for dt_i in range(DT):
                            nc.tensor.matmul(
                                psq[:],
                                wq_sb[:, dt_i, g * P:(g + 1) * P],
                                xT_sb[:, dt_i, sl],
                                start=(dt_i == 0),
                                stop=(dt_i == DT - 1),
                            )
                        nc.vector.tensor_copy(qT_sb[:, g, sl], psq[:])

                qproj(0)
                for tqt in range(NTQ):
                    if tqt + 1 < NTQ:
                        load_stage(tqt + 1)
                    attention(0, tqt)
                    if tqt + 1 < NTQ:
                        qproj(tqt + 1)
                    attention(1, tqt)
                while pending:
                    epilogue(pending.popleft())

            if repeat == 1:
                body()
            else:
                tc.For_i_unrolled(0, repeat, 1, body, max_unroll=1)

    nc.compile()
    return nc


def prep_in_maps(x, k, v, Wq, variant: str = VARIANT):
    """Build the 8 per-core input maps from full inputs (host-side numpy)."""
    x = np.asarray(x, dtype=np.float32)
    k = np.asarray(k, dtype=np.float32)
    v = np.asarray(v, dtype=np.float32)
    Wq = np.asarray(Wq, dtype=np.float32)

    import ml_dtypes

    # mask2[kk, u, j] over the 256-wide mixed window of a diagonal pair
    # (uint8 0xFF = visible, ANDed onto the fp8 et bytes).
    kk = np.arange(P)[:, None, None]
    uu = np.arange(2)[None, :, None]
    jj = np.arange(2 * P)[None, None, :]
    mask2 = np.where(kk + P * uu <= jj, np.uint8(0xFF), np.uint8(0))
    ident = np.eye(P, dtype=np.float32)

    in_maps = []
    for c in range(N_CORES):
        b = c // (N_CORES // B)
        grp = c % (N_CORES // B)
        heads = slice(HPC * grp, HPC * (grp + 1))
        cols = slice(EPC * grp, EPC * (grp + 1))

        xT_c = x[b].T                                   # [D, T]
        wqT_c = Wq[cols, :].T                           # [D, EPC]
        kT_c = np.zeros((P, EG, T), dtype=np.float32)
        for lh in range(HPC):
            kT_c[DH * (lh % 2):DH * (lh % 2) + DH, lh // 2, :] = \
                k[b, HPC * grp + lh].T
        vv = v[b, heads]                                # [HPC, T, DH]
        vO_c = np.ones((P, NTKB, HPC, VPAD), dtype=np.float32)
        vO_c[:, :, :, :DH] = vv.reshape(HPC, NTKB, P, DH).transpose(2, 1, 0, 3)
        xres_c = np.ascontiguousarray(
            x[b][:, cols].reshape(NTKB, P, EPC).transpose(1, 0, 2)
        )
        in_maps.append({
            "xT": _host_cast(xT_c, variant),
            "wqT": _host_cast(wqT_c, variant),
            "kT": _host_cast(kT_c, variant),
            "vO": vO_c.astype(ml_dtypes.float8_e4m3),
            "xres": xres_c,
            "mask2": mask2,
            "ident": ident,
        })
    return in_maps


def gather_output(results):
    """Assemble full [B, T, D] output from 8 per-core [T, EPC] slices."""
    y = np.empty((B, T, D), dtype=np.float32)
    for c in range(N_CORES):
        b = c // (N_CORES // B)
        grp = c % (N_CORES // B)
        y[b, :, EPC * grp:EPC * (grp + 1)] = results[c]["y"]
    return y


_NC_CACHE = {}


def kernel(x, k, v, Wq):
    key = (VARIANT, 1)
    if key not in _NC_CACHE:
        _NC_CACHE[key] = build_nc(VARIANT, repeat=1)
    nc = _NC_CACHE[key]
    in_maps = prep_in_maps(x, k, v, Wq, VARIANT)
    res = run_bass_kernel_spmd(nc, in_maps, core_ids=list(range(N_CORES)))
    return gather_output(res.results)


# revision 21
# speedup vs baseline: 1.1215x; 1.1215x over previous
"""Trainium2 Bass kernel for nn_DecoderHead (B=2, T=2048, D=1024, H=16, DH=64).

y = x + softmax_causal((x @ Wq.T) split to heads @ k^T / sqrt(D)) @ v

Sharding: 8 cores = 2 (batch) x 4 (head groups of 4 heads). Each core computes
its batch's q-projection for its 256 output features (Wq column-sharded by
head), causal attention for its 4 heads, adds the residual slice, and writes a
[T, 256] slice; the host concatenates slices (the all-gather over the
head-split d dim is a free host-side assembly).

Per-core dataflow (all matmul contractions on the PE partition axis):
  qT[e, t]   = sum_d WqT[d, e] * xT[d, t]         (q projection, transposed)
  sT[tk, tq] = sum_dh kT_h[dh, tk] * qT_h[dh, tq] (scores, transposed; the two
               heads of a pair are emitted INTERLEAVED so the PE runs them
               concurrently in distinct 64-row groups -> 2x score rate)
  eT         = exp(sT / 32), causal-masked        (ACT exp on the visible
               column range only; DVE mask-mul only on the two mixed
               128-col diagonal blocks of each diagonal pair)
  oT[dh', tq]= sum_tk vO[tk, dh'] * eT[tk, tq]    (vO = [v | ones]; row 64
               accumulates the denominator; streams only visible columns)
  y[tq, dh]  = transpose(oT) / denom + x_res      (PE transpose into one PSUM
                                                   bank, fused DVE epilogue)

All matmul operands are bf16 (fp32 PSUM accumulate): standalone LDWEIGHTS +
FWL make the weight loads hide behind the streams, and row-group pairing
works (fp32r self-loading matmuls serialize the array).
"""

import os
from collections import deque

import numpy as np

import concourse.bass as bass
import concourse.mybir as mybir
import concourse.tile as tile
from concourse import bacc
from concourse.alu_op_type import AluOpType
from concourse.bass_utils import run_bass_kernel_spmd

# Problem shape (hardcoded per the harness contract).
B, T, D, H = 2, 2048, 1024, 16
DH = D // H          # 64
N_CORES = 8
HPC = H // (N_CORES // B)   # heads per core = 4
EPC = HPC * DH       # output features per core = 256
P = 128              # SBUF partitions
TQ = 512             # query-tile width (matmul moving-dim)
NTQ = T // TQ        # 4
NTKB = T // P        # 16 key blocks of 128
DT = D // P          # 8 contraction tiles for the q projection
EG = EPC // P        # 2 head-pair groups of 128 e-rows
SCALE = 1.0 / np.sqrt(np.float32(D))   # 1/32 (reference scales by sqrt(d))

F32 = mybir.dt.float32
FP8 = mybir.dt.float8e4
U8 = mybir.dt.uint8
I8 = mybir.dt.int8
VPAD = 80            # vO innermost pad: DoubleRow LDW needs Ko step % 16 == 0
LOG2E = 1.4426950408889634
# Schraudolph exp for fp8e4m3 out: bits = round(8*(x*SCALE*log2e - c) + 56)
SCH_A = float(1.0 / np.sqrt(np.float32(D))) * LOG2E * 8.0
SCH_B = 56.0 - 8.0 * 0.0430

# Matmul operand dtype: bf16 (default; full PE rate + hideable weight loads),
# fp32r (fp32 w/ 11-bit mantissa), fp32 (exact, 1/4 rate).
VARIANT = os.environ.get("DH_VARIANT", "bf16")
# Route head-0 non-diagonal exps to the DVE (Schraudolph bit-trick) to
# offload the saturated ACT engine.
DVE_EXP = os.environ.get("DH_DVE_EXP", "1") == "1"


def _mm_dt(variant):
    return {
        "fp32": mybir.dt.float32,
        "fp32r": mybir.dt.float32r,
        "bf16": mybir.dt.bfloat16,
    }[variant]


def _np_round_fp32r(a: np.ndarray) -> np.ndarray:
    """Round fp32 to the fp32r value set: 11-bit mantissa, RNE, low 12 bits 0."""
    u = a.astype(np.float32).view(np.uint32)
    lsb = (u >> np.uint32(12)) & np.uint32(1)
    r = (u + np.uint32(0x7FF) + lsb) & np.uint32(0xFFFFF000)
    return r.view(np.float32)


def _host_cast(a: np.ndarray, variant: str) -> np.ndarray:
    a = np.ascontiguousarray(a, dtype=np.float32)
    if variant == "fp32r":
        return _np_round_fp32r(a)
    if variant == "bf16":
        import ml_dtypes
        return a.astype(ml_dtypes.bfloat16)
    return a


def build_nc(variant: str = VARIANT, repeat: int = 1):
    """Build the per-core SPMD Bass program. `repeat` wraps the body in a
    hardware loop (timing only)."""
    mdt = _mm_dt(variant)
    # oT / identity dtype: the PE transpose requires out/lhsT dtype match and
    # pst is fp32 PSUM, so these stay fp32 in every variant.
    odt = F32
    nc = bacc.Bacc(
        "TRN2", target_bir_lowering=False, debug=False, num_devices=N_CORES
    )

    xT = nc.dram_tensor("xT", [D, T], mdt, kind="ExternalInput").ap()
    wqT = nc.dram_tensor("wqT", [D, EPC], mdt, kind="ExternalInput").ap()
    kT = nc.dram_tensor("kT", [P, EG, T], mdt, kind="ExternalInput").ap()
    vO = nc.dram_tensor("vO", [P, NTKB, HPC, VPAD], FP8, kind="ExternalInput").ap()
    xres = nc.dram_tensor("xres", [P, T // P, EPC], F32, kind="ExternalInput").ap()
    # mask2[k, u, j]: causal mask (0x00/0xFF bytes, ANDed onto fp8 et) for the
    # 256-wide mixed window of a diagonal pair: u=0 block -> (k <= j) for
    # j<128 else 1; u=1 block -> 0 for j<128 else (k <= j-128).
    mask2 = nc.dram_tensor("mask2", [P, 2, 2 * P], U8, kind="ExternalInput").ap()
    ident = nc.dram_tensor("ident", [P, P], F32, kind="ExternalInput").ap()
    y = nc.dram_tensor("y", [T, EPC], F32, kind="ExternalOutput").ap()

    with tile.TileContext(nc) as tc:
        with (
            tc.tile_pool(name="const", bufs=1) as cpool,
            tc.tile_pool(name="xq", bufs=1) as xqpool,
            tc.tile_pool(name="work", bufs=6) as wpool,
            tc.tile_pool(name="epi", bufs=2) as epool,
            tc.tile_pool(name="ps_s", bufs=3, space="PSUM") as ps_s,
            tc.tile_pool(name="ps_o", bufs=2, space="PSUM") as ps_o,
        ):
            def body(_iv=None):
                # ---- tiles -------------------------------------------------
                id_sb = cpool.tile([P, P], F32, name="id_sb", tag="id_sb")
                mk_sb = cpool.tile([P, 2, 2 * P], U8, name="mk_sb", tag="mk_sb")
                wq_sb = xqpool.tile([P, DT, EPC], mdt, name="wq_sb", tag="wq_sb")
                xT_sb = xqpool.tile([P, DT, T], mdt, name="xT_sb", tag="xT_sb")
                kT_sb = cpool.tile([P, EG, T], mdt, name="kT_sb", tag="kT_sb")
                vO_sb = cpool.tile([P, NTKB, HPC, VPAD], FP8, name="vO_sb",
                                   tag="vO_sb")
                xr_sb = cpool.tile([P, T // P, EPC], F32, name="xr_sb",
                                   tag="xr_sb")
                qT_sb = xqpool.tile([P, EG, T], mdt, name="qT_sb", tag="qT_sb")

                # ---- stage-0 loads ----------------------------------------
                nc.sync.dma_start(id_sb[:], ident[:])
                for dt_i in range(DT):
                    nc.sync.dma_start(
                        wq_sb[:, dt_i, :], wqT[dt_i * P:(dt_i + 1) * P, :]
                    )

                def load_stage(c):
                    """Inputs first needed by tq-tile c."""
                    sl = bass.ts(c, TQ)
                    for dt_i in range(DT):
                        nc.sync.dma_start(
                            xT_sb[:, dt_i, sl], xT[dt_i * P:(dt_i + 1) * P, sl]
                        )
                    for g in range(EG):
                        nc.sync.dma_start(kT_sb[:, g, sl], kT[:, g, sl])
                    nc.sync.dma_start(
                        vO_sb[:, 4 * c:4 * (c + 1)], vO[:, 4 * c:4 * (c + 1)]
                    )
                    nc.sync.dma_start(
                        xr_sb[:, 4 * c:4 * (c + 1)], xres[:, 4 * c:4 * (c + 1)]
                    )

                load_stage(0)
                nc.sync.dma_start(mk_sb[:], mask2[:])

                # Warm-up while stage-0 DMA streams: prime the ACT exp table
                # and keep PE busy so the HAM clock-gate opens (dummy work on
                # the identity tile; results unused).
                warm_et = wpool.tile([P, P], F32, name="warm_et", tag="warm")
                psw = ps_o.tile([P, P], F32, name="psw", tag="o")
                for w in range(12):
                    nc.tensor.matmul(
                        psw[:], id_sb[:], id_sb[:], start=True, stop=True,
                    )
                nc.scalar.activation(
                    warm_et[:], psw[:],
                    mybir.ActivationFunctionType.Exp, scale=0.01,
                )

                pending = deque()

                def epilogue_start(h, tqt, pso_t):
                    oT = epool.tile([DH + 1, TQ], F32, name="oT", tag="oT",
                                    bufs=4)
                    nc.vector.tensor_copy(oT[:], pso_t[:])
                    return (h, tqt, oT)

                def epilogue(state):
                    h, tqt, oT = state
                    ysb = epool.tile([P, 4, DH], F32, name="ysb", tag="ysb")
                    pst = ps_o.tile([P, 4, DH + 1], F32, name="pst", tag="o")
                    for j in range(4):
                        nc.tensor.transpose(
                            pst[:, j, :],
                            oT[:, j * P:(j + 1) * P],
                            id_sb[0:DH + 1, 0:DH + 1],
                        )
                    rc = epool.tile([P, 4], F32, name="rc", tag="rc", bufs=4)
                    nc.vector.reciprocal(rc[:], pst[:, :, DH])
                    for j in range(4):
                        nc.vector.scalar_tensor_tensor(
                            ysb[:, j, :],
                            pst[:, j, 0:DH],
                            rc[:, j:j + 1],
                            xr_sb[:, 4 * tqt + j, h * DH:(h + 1) * DH],
                            AluOpType.mult,
                            AluOpType.add,
                        )
                    ydst = y[tqt * TQ:(tqt + 1) * TQ, h * DH:(h + 1) * DH]
                    nc.sync.dma_start(
                        ydst.rearrange("(j p) c -> p j c", p=P), ysb[:]
                    )

                def attention(hp, tqt):
                    g = hp
                    ntk = 4 * (tqt + 1)
                    npairs = ntk // 2
                    tq0 = tqt * TQ
                    pso2 = [
                        ps_o.tile([DH + 1, TQ], F32, name=f"pso{i}", tag="o")
                        for i in range(2)
                    ]

                    def vis_of(tkb):
                        # first visible query column (within the TQ tile) for
                        # key block tkb; columns below are fully masked.
                        return max(0, P * (tkb - 4 * tqt))

                    def emit_pv(p_et2, p_pair, last=False):
                        # One DoubleRow matmul per head covers both key blocks
                        # of the pair (Ko=2 contraction halves).
                        vis = vis_of(2 * p_pair)
                        for i in range(2):
                            nc.tensor.matmul(
                                pso2[i][:, vis:],
                                vO_sb[:, 2 * p_pair:2 * p_pair + 2,
                                      2 * hp + i, 0:DH + 1],
                                p_et2[i][:, :, vis:],
                                start=(p_pair == 0),
                                stop=last,
                                perf_mode=mybir.MatmulPerfMode.DoubleRow,
                                skip_group_check=True,
                            )

                    prev = None
                    for pair in range(npairs):
                        diag = 2 * pair >= 4 * tqt   # this pair straddles the
                        m0 = 2 * pair - 4 * tqt      # causal diagonal
                        vis0 = vis_of(2 * pair)
                        et2 = []
                        pssc2 = [
                            ps_s.tile([P, 2, TQ], F32, name=f"pssc{i}", tag="s")
                            for i in range(2)
                        ]
                        # scores: interleave the two heads (i) inside the key
                        # block loop (u) so adjacent matmuls target distinct
                        # 64-row PE groups and run concurrently.
                        for u in range(2):
                            tkb = 2 * pair + u
                            vis = vis_of(tkb)
                            for i in range(2):
                                bp = DH * i
                                nc.tensor.matmul(
                                    pssc2[i][:, u, vis:],
                                    kT_sb[bp:bp + DH, g,
                                          tkb * P:(tkb + 1) * P],
                                    qT_sb[bp:bp + DH, g, tq0 + vis:tq0 + TQ],
                                    start=True,
                                    stop=True,
                                    skip_group_check=True,
                                )
                        for i in range(2):
                            et = wpool.tile([P, 2, TQ], FP8,
                                            name=f"et{i}", tag="et")
                            # NOTE: exp covers [vis0:] for BOTH u (the u=1
                            # block's extra 128 cols are zeroed by the mask)
                            # so the DoubleRow stream never reads garbage.
                            if DVE_EXP and not diag and i == 0:
                                nc.vector.tensor_scalar(
                                    et[:].bitcast(I8), pssc2[i][:],
                                    SCH_A, SCH_B,
                                    AluOpType.mult, AluOpType.add,
                                )
                            else:
                                nc.scalar.activation(
                                    et[:, :, vis0:], pssc2[i][:, :, vis0:],
                                    mybir.ActivationFunctionType.Exp,
                                    scale=float(SCALE),
                                )
                            if diag:
                                # AND-mask the 256-wide mixed window (covers
                                # the two mixed 128-col diagonal blocks and
                                # zeroes the u=1 head-start strip). Runs on
                                # the otherwise-idle GpSimd engine.
                                w0 = P * m0
                                nc.vector.tensor_tensor(
                                    et[:, :, w0:w0 + 2 * P].bitcast(U8),
                                    et[:, :, w0:w0 + 2 * P].bitcast(U8),
                                    mk_sb[:],
                                    AluOpType.bitwise_and,
                                )
                            et2.append(et)
                        if prev is not None:
                            emit_pv(*prev)
                        prev = (et2, pair)
                        if pending and pair < 2:
                            epilogue(pending.popleft())
                    emit_pv(*prev, last=True)
                    for i in range(2):
                        pending.append(epilogue_start(2 * hp + i, tqt, pso2[i]))

                # ---- main schedule: staged loads; qproj for tile c+1 is
                # emitted between the two head-pairs of attention tile c so
                # its matmuls fill PE stall slots while ACT/DVE chew on exp.
                def qproj(tqc):
                    sl = bass.ts(tqc, TQ)
                    for g in range(EG):
                        psq = ps_s.tile([P, TQ], F32, name="psq", tag="s")
                        for dt_i in range(DT):
                            nc.tensor.matmul(
                                psq[:],
                                wq_sb[:, dt_i, g * P:(g + 1) * P],
                                xT_sb[:, dt_i, sl],
                                start=(dt_i == 0),
                                stop=(dt_i == DT - 1),
                            )
                        nc.vector.tensor_copy(qT_sb[:, g, sl], psq[:])

                qproj(0)
                for tqt in range(NTQ):
                    if tqt + 1 < NTQ:
                        load_stage(tqt + 1)
                    attention(0, tqt)
                    if tqt + 1 < NTQ:
                        qproj(tqt + 1)
                    attention(1, tqt)
                while pending:
                    epilogue(pending.popleft())

            if repeat == 1:
                body()
            else:
                tc.For_i_unrolled(0, repeat, 1, body, max_unroll=1)

    nc.compile()
    return nc


def prep_in_maps(x, k, v, Wq, variant: str = VARIANT):
    """Build the 8 per-core input maps from full inputs (host-side numpy)."""
    x = np.asarray(x, dtype=np.float32)
    k = np.asarray(k, dtype=np.float32)
    v = np.asarray(v, dtype=np.float32)
    Wq = np.asarray(Wq, dtype=np.float32)

    import ml_dtypes

    # mask2[kk, u, j] over the 256-wide mixed window of a diagonal pair
    # (uint8 0xFF = visible, ANDed onto the fp8 et bytes).
    kk = np.arange(P)[:, None, None]
    uu = np.arange(2)[None, :, None]
    jj = np.arange(2 * P)[None, None, :]
    mask2 = np.where(kk + P * uu <= jj, np.uint8(0xFF), np.uint8(0))
    ident = np.eye(P, dtype=np.float32)

    in_maps = []
    for c in range(N_CORES):
        b = c // (N_CORES // B)
        grp = c % (N_CORES // B)
        heads = slice(HPC * grp, HPC * (grp + 1))
        cols = slice(EPC * grp, EPC * (grp + 1))

        xT_c = x[b].T                                   # [D, T]
        wqT_c = Wq[cols, :].T                           # [D, EPC]
        kT_c = np.zeros((P, EG, T), dtype=np.float32)
        for lh in range(HPC):
            kT_c[DH * (lh % 2):DH * (lh % 2) + DH, lh // 2, :] = \
                k[b, HPC * grp + lh].T
        vv = v[b, heads]                                # [HPC, T, DH]
        vO_c = np.ones((P, NTKB, HPC, VPAD), dtype=np.float32)
        vO_c[:, :, :, :DH] = vv.reshape(HPC, NTKB, P, DH).transpose(2, 1, 0, 3)
        xres_c = np.ascontiguousarray(
            x[b][:, cols].reshape(NTKB, P, EPC).transpose(1, 0, 2)
        )
        in_maps.append({
            "xT": _host_cast(xT_c, variant),
            "wqT": _host_cast(wqT_c, variant),
            "kT": _host_cast(kT_c, variant),
            "vO": vO_c.astype(ml_dtypes.float8_e4m3),
            "xres": xres_c,
            "mask2": mask2,
            "ident": ident,
        })
    return in_maps


def gather_output(results):
    """Assemble full [B, T, D] output from 8 per-core [T, EPC] slices."""
    y = np.empty((B, T, D), dtype=np.float32)
    for c in range(N_CORES):
        b = c // (N_CORES // B)
        grp = c % (N_CORES // B)
        y[b, :, EPC * grp:EPC * (grp + 1)] = results[c]["y"]
    return y


_NC_CACHE = {}


def kernel(x, k, v, Wq):
    key = (VARIANT, 1)
    if key not in _NC_CACHE:
        _NC_CACHE[key] = build_nc(VARIANT, repeat=1)
    nc = _NC_CACHE[key]
    in_maps = prep_in_maps(x, k, v, Wq, VARIANT)
    res = run_bass_kernel_spmd(nc, in_maps, core_ids=list(range(N_CORES)))
    return gather_output(res.results)
